# revision 25
# baseline (speedup 1.0000x reference)
"""Bidirectional LSTM encoder (nn_BiEncode) as a Bass/Tile kernel on 8 trn2 cores.

Sharding: direction-split x batch-split. Cores 0-3 run the LEFT (forward-time)
direction on batch shards 0-3 (512 rows each); cores 4-7 run the RIGHT
direction (time-reversed input, handled host-side) on the same batch shards.
Every core runs the identical SPMD program; direction differences live
entirely in the data it is fed (weights + time-reversed x).

Device layout: everything is kept "transposed" (feature dim on partitions,
batch on the free dim) so the scan needs no on-chip transposes:
  x fed as xT[t, i, b], weights as W^T, h/c as [H, B] tiles, output written
  as yT[t, h, b] and un-transposed on the host.

Per timestep the full gate pre-activation g^T[4H, B] is computed as 12
PSUM-accumulated matmuls per 128-row gate tile (8 k-tiles of x-projection +
4 k-tiles of the recurrent term) -- the input projection is fused into the
scan, so no pre-activation tensor is ever materialized. ACT applies
sigmoid/tanh straight out of PSUM; DVE does the cell update.

Dtype choice (all measured on hardware): every candidate runs the PE at
1 column/cycle nominal, but the steady-state matmul issue cadence differs:
fp16 215.8ns vs f32r 226.7ns vs bf16 259ns per [128k x 128m x 512n] matmul
(f32r appears to hit a ~97% power-throttle duty limit that fp16 avoids).
fp16 operands keep a 10-bit mantissa (tf32-class, rel err ~1.3e-3 end to
end) and halve all DMA traffic, so x/w/h/y are all fp16 with fp32 PSUM
accumulation and an fp32 cell state. fp8 was evaluated and rejected:
e4m3 quantization alone gives 1.4e-1 relative error (vs the 2e-2 gate),
and DoubleRow fp8 measures 1.0 cyc/col effective (2x MACs per instr),
so the accuracy-viable 3-term residual scheme would be 1.5x SLOWER.

The kernel is PE-bound at the flat 215.8ns cadence (MFU ~95%); the only
slack is the DMA-bound startup ramp, so the startup transfers are split
across three DMA queues with fine-grained first chunks (see below).
"""

import os

import numpy as np

FRAME_LENGTH = 26
HIDDEN = 512
INPUT = 1024
BATCH = 2048

NCORES = 8
NSHARD = 4                 # batch shards per direction group
BC = BATCH // NSHARD       # 512 batch rows per core

P = 128
KI = INPUT // P            # 8  k-tiles for the input projection
KH = HIDDEN // P           # 4  k-tiles for the recurrent matmul
NJ = HIDDEN // P           # 4  hidden chunks
NM = 4 * HIDDEN // P       # 16 gate m-tiles

# "fp16": fp16 storage+PE operands (half DMA/SBUF, fp32 PSUM accumulation,
#         10-bit mantissa ~ tf32 class, and ~5% faster PE cadence than f32r --
#         fp16 matmuls run below the power-throttle duty limit f32r hits)
# "f32r": fp32 storage, PE in float32r (full-rate at N>=256, ~tf32 precision)
# "bf16": bf16 storage+PE (half DMA/SBUF), fp32 PSUM accumulation
MM_MODE = os.environ.get("BASS_LSTM_MM", "fp16")
W16 = os.environ.get("BASS_LSTM_W16", "1") == "1"   # ship weights fp16
Y16 = os.environ.get("BASS_LSTM_Y16", "1") == "1"   # output y as fp16

_CACHE = {}


def _build(T, Bc, mode, w16, y16):
    import concourse.mybir as mybir
    import concourse.tile as tile
    from concourse import bacc

    dt = mybir.dt
    AF = mybir.ActivationFunctionType

    # matmul-operand storage dtype; the BIR verifier requires fp32r matmul
    # inputs to be produced as fp32r, so x/w/h carry it end-to-end
    io_dt = {"bf16": dt.bfloat16, "fp16": dt.float16}.get(mode, dt.float32r)
    w16 = w16 and mode == "f32r"
    w_dt = dt.float16 if w16 else io_dt
    y_dt = dt.float16 if y16 else io_dt
    y_copy = y_dt != io_dt

    nc = bacc.Bacc("TRN2", target_bir_lowering=False, debug=False,
                   num_devices=NCORES)
    # All inputs are host-laid-out to match the SBUF tiles exactly, so every
    # DMA moves maximally contiguous runs per partition (large descriptors:
    # x 16KB/partition per step, w_ih 8KB/partition per j-chunk). 1-2KB
    # descriptors measurably halve per-queue DMA throughput.
    xT = nc.dram_tensor("xT", [T, P, KI, Bc], io_dt,
                        kind="ExternalInput").ap()
    w_ih = nc.dram_tensor("w_ih", [P, NJ, KI, 4 * P], w_dt,
                          kind="ExternalInput").ap()
    w_hh = nc.dram_tensor("w_hh", [P, NJ, KH, 4 * P], w_dt,
                          kind="ExternalInput").ap()
    bias = nc.dram_tensor("bias", [P, NM], dt.float32, kind="ExternalInput").ap()
    yT = nc.dram_tensor("yT", [T, HIDDEN, Bc], y_dt, kind="ExternalOutput").ap()

    with tile.TileContext(nc) as tc:
        with tc.tile_pool(name="wpool", bufs=1) as wp, \
             tc.tile_pool(name="xpool", bufs=2) as xp, \
             tc.tile_pool(name="work", bufs=1) as wk, \
             tc.tile_pool(name="psum", bufs=2, space="PSUM") as pp:

            # Startup-latency plan (the kernel is PE-bound; all slack is in
            # the DMA-bound ramp): DMA trigger issue costs ~0.64us each on an
            # engine sequencer, so the startup transfers are spread across
            # four engines' HWDGE queues to issue in parallel:
            #   sync:   w_ih fp16 (j0 per-k slices first, then j1..j3)
            #   scalar: w_hh fp16
            #   gpsimd: t=0 x slices + t=1 x prefetch
            # (DVE does the fp16->f32r upcasts but cannot issue DMAs)
            # Subtile dependency tracking lets the first matmul start once
            # x[k=0] and the k=0 slice of the j0 weight upcast land.
            # bias rides the gpsimd queue: keeps the sync queue's first
            # trigger for the j0 weights; lands ~11us, first ACT needs it
            # ~16.5us
            bias_sb = wp.tile([P, NM], dt.float32, tag="bias")
            nc.gpsimd.dma_start(out=bias_sb, in_=bias[:, :])

            def upcast(dst_f32r, src_f16):
                # fp16 -> f32r convert (f32r-typed output keeps the BIR
                # verifier happy about fp32r matmul operands)
                nc.vector.tensor_copy(dst_f32r, src_f16)

            # t=0 x slices ride the scalar queue (ahead of w_hh, which isn't
            # needed until t=1) so they land while the sync queue streams j0
            # weights; the gpsimd SWDGE queue spins up ~2.5us later and only
            # carries the t=1 prefetch. Separate per-k TILES (not slices of
            # one tile) so the first matmul's dependency is only the k=0
            # slice pair -- slice-DMAs into one tile were observed to make
            # the first matmul wait for all eight.
            if dt.size(io_dt) == 2:
                xt0k = []
                for k in range(KI):
                    xk = xp.tile([P, Bc], io_dt, tag=f"x0_{k}", bufs=1)
                    nc.scalar.dma_start(out=xk, in_=xT[0, :, k, :])
                    xt0k.append(xk)
            else:
                # f32r fallback: no SBUF room for separate tiles
                xt0 = xp.tile([P, KI, Bc], io_dt, tag="x")
                for k in range(KI):
                    nc.scalar.dma_start(out=xt0[:, k, :], in_=xT[0, :, k, :])
                xt0k = [xt0[:, k, :] for k in range(KI)]

            w_ih_sb = []          # [j][k] -> [P, 4P] AP
            w_hh_sb = []          # [j] -> [P, KH, 4P] tile
            if w16:
                # j=0 flows per-k for minimum first-matmul latency
                wt0 = wp.tile([P, KI, 4 * P], io_dt, tag="wih0")
                for k in range(KI):
                    wfk = wp.tile([P, 4 * P], dt.float16, tag="wsk", bufs=4)
                    nc.sync.dma_start(out=wfk, in_=w_ih[:, 0, k, :])
                    upcast(wt0[:, k, :], wfk)
                w_ih_sb.append([wt0[:, k, :] for k in range(KI)])
                for j in range(1, NJ):
                    wf = wp.tile([P, KI, 4 * P], dt.float16, tag="ws", bufs=2)
                    nc.sync.dma_start(out=wf, in_=w_ih[:, j, :, :])
                    wt = wp.tile([P, KI, 4 * P], io_dt, tag=f"wih{j}")
                    upcast(wt, wf)
                    w_ih_sb.append([wt[:, k, :] for k in range(KI)])
                for j in range(NJ):
                    wf = wp.tile([P, KH, 4 * P], dt.float16, tag="whs", bufs=2)
                    nc.scalar.dma_start(out=wf, in_=w_hh[:, j, :, :])
                    wt = wp.tile([P, KH, 4 * P], io_dt, tag=f"whh{j}")
                    upcast(wt, wf)
                    w_hh_sb.append(wt)
            else:
                # j0 per-k (first matmul waits only 128KB), j1-3 as
                # half-chunks (4KB-contiguous descriptors, finer pipelining).
                # j1 rides the scalar queue right behind the t=0 x slices so
                # it lands before the PE finishes chewing j0; j2/j3 follow j0
                # on the sync queue; w_hh (not needed until t=1, ~50us in)
                # queues last on scalar.
                h2 = KI // 2
                for j in range(NJ):
                    if j == 0:
                        row = []
                        for k in range(KI):
                            w0k = wp.tile([P, 4 * P], io_dt, tag=f"w0_{k}")
                            nc.sync.dma_start(out=w0k, in_=w_ih[:, 0, k, :])
                            row.append(w0k)
                        w_ih_sb.append(row)
                    else:
                        wt = wp.tile([P, KI, 4 * P], io_dt, tag=f"wihf{j}")
                        eng = nc.scalar if j == 1 else nc.sync
                        eng.dma_start(out=wt[:, :h2, :],
                                      in_=w_ih[:, j, :h2, :])
                        eng.dma_start(out=wt[:, h2:, :],
                                      in_=w_ih[:, j, h2:, :])
                        w_ih_sb.append([wt[:, k, :] for k in range(KI)])
                for j in range(NJ):
                    wt = wp.tile([P, KH, 4 * P], io_dt, tag=f"whh{j}")
                    nc.scalar.dma_start(out=wt, in_=w_hh[:, j, :, :])
                    w_hh_sb.append(wt)
            # prefetch t=1's x on the gpsimd engine's queue
            xt1 = None
            if T > 1:
                xt1 = xp.tile([P, KI, Bc], io_dt, tag="x")
                nc.gpsimd.dma_start(out=xt1, in_=xT[1])

            # h0 = c0 = 0, so step 0 skips the recurrent matmuls and the
            # f*c term entirely -- no initial state tiles needed (memset
            # can't produce float32r anyway).
            h_cur, c_cur = [], []

            GATE_FUNCS = (AF.Sigmoid, AF.Sigmoid, AF.Tanh, AF.Sigmoid)

            for t in range(T):
                if t == 0:
                    xt = None
                elif t == 1:
                    xt = xt1
                else:
                    xt = xp.tile([P, KI, Bc], io_dt, tag="x")
                    nc.sync.dma_start(out=xt, in_=xT[t])

                h_next, c_next = [], []
                for j in range(NJ):
                    acts = {}
                    # t=0: c0=0 makes the f-gate's contribution f*c0 vanish,
                    # so its whole matmul chain + sigmoid are skipped
                    for gi in ((0, 2, 3) if t == 0 else range(4)):
                        m = gi * NJ + j
                        ps = pp.tile([P, Bc], dt.float32, tag=f"ps{gi}")
                        for k in range(KI):
                            nc.tensor.matmul(
                                ps, lhsT=w_ih_sb[j][k][:, gi * P:(gi + 1) * P],
                                rhs=(xt0k[k] if t == 0 else xt[:, k, :]),
                                start=(k == 0),
                                stop=(t == 0 and k == KI - 1))
                        if t > 0:
                            for k in range(KH):
                                nc.tensor.matmul(
                                    ps, lhsT=w_hh_sb[j][:, k, gi * P:(gi + 1) * P],
                                    rhs=h_cur[k],
                                    start=False, stop=(k == KH - 1))
                        gt = wk.tile([P, Bc], dt.float32, tag=f"g{gi}",
                                     bufs=2)
                        nc.scalar.activation(gt, ps, GATE_FUNCS[gi],
                                             bias=bias_sb[:, m:m + 1])
                        acts[gi] = gt
                    i_t, g_t, o_t = acts[0], acts[2], acts[3]
                    cn = wk.tile([P, Bc], dt.float32, tag=f"c{j}")
                    if t == 0:
                        nc.vector.tensor_mul(cn, i_t, g_t)
                    else:
                        f_t = acts[1]
                        u = wk.tile([P, Bc], dt.float32, tag="u")
                        nc.vector.tensor_mul(u, i_t, g_t)
                        v = wk.tile([P, Bc], dt.float32, tag="v")
                        nc.vector.tensor_mul(v, f_t, c_cur[j])
                        nc.vector.tensor_add(cn, u, v)
                    th = wk.tile([P, Bc], dt.float32, tag="th")
                    nc.scalar.activation(th, cn, AF.Tanh)
                    hn = wk.tile([P, Bc], io_dt, tag=f"h{j}", bufs=2)
                    nc.vector.tensor_mul(hn, o_t, th)
                    if y_copy:
                        yh = wk.tile([P, Bc], dt.float16, tag="yh")
                        nc.vector.tensor_copy(yh, hn)
                        nc.sync.dma_start(out=yT[t, j * P:(j + 1) * P, :],
                                          in_=yh)
                    else:
                        nc.sync.dma_start(out=yT[t, j * P:(j + 1) * P, :],
                                          in_=hn)
                    h_next.append(hn)
                    c_next.append(cn)
                h_cur, c_cur = h_next, c_next

    nc.compile()
    return nc


def _get_nc(T=FRAME_LENGTH, Bc=BC, mode=MM_MODE, w16=W16, y16=Y16):
    key = (T, Bc, mode, w16, y16)
    if key not in _CACHE:
        _CACHE[key] = _build(T, Bc, mode, w16, y16)
    return _CACHE[key]


def _prep_inputs(embed_feats, w_ih_l, w_hh_l, b_ih_l, b_hh_l,
                 w_ih_r, w_hh_r, b_ih_r, b_hh_r, mode, w16):
    import ml_dtypes

    io_np = {"bf16": ml_dtypes.bfloat16,
             "fp16": np.float16}.get(mode, np.float32)
    w_np = np.float16 if (w16 and mode == "f32r") else io_np
    T = embed_feats.shape[1]

    w = {
        0: (np.asarray(w_ih_l), np.asarray(w_hh_l),
            np.asarray(b_ih_l) + np.asarray(b_hh_l)),
        1: (np.asarray(w_ih_r), np.asarray(w_hh_r),
            np.asarray(b_ih_r) + np.asarray(b_hh_r)),
    }
    x = np.asarray(embed_feats)

    # j-major column permutation of the 4H gate dim: block j holds the four
    # gates' columns for hidden chunk j, so each j-chunk loads contiguously
    j_idx, g_idx, c_idx = np.meshgrid(
        np.arange(NJ), np.arange(4), np.arange(P), indexing="ij")
    perm = (g_idx * (NJ * P) + j_idx * P + c_idx).reshape(-1)

    def dev_w(wmat, kt):
        # [IN, 4H] -> j-major column perm -> [P, NJ, kt, 4P] (the on-chip
        # tile layout, so each j-chunk DMA is fully contiguous per partition)
        wT = wmat.T[:, perm]                      # [IN, 4H] j-major cols
        wr = wT.reshape(kt, P, NJ, 4 * P)         # (k p) rows
        return np.ascontiguousarray(wr.transpose(1, 2, 0, 3)).astype(w_np)

    in_maps = []
    for c in range(NCORES):
        d, s = c // NSHARD, c % NSHARD
        xs = x[s * BC:(s + 1) * BC]
        if d == 1:
            xs = xs[:, ::-1]
        # [Bc, T, IN] -> [T, P, KI, Bc] (on-chip step-tile layout)
        xT = np.ascontiguousarray(
            xs.transpose(1, 2, 0).reshape(T, KI, P, BC)
            .transpose(0, 2, 1, 3)).astype(io_np)
        bias = np.ascontiguousarray(
            w[d][2].astype(np.float32).reshape(NM, P).T)
        in_maps.append({"xT": xT, "w_ih": dev_w(w[d][0], KI),
                        "w_hh": dev_w(w[d][1], KH), "bias": bias})
    return in_maps, T


def _run(inputs, mode=MM_MODE, trace=False, trace_kwargs=None):
    from concourse.bass_utils import run_bass_kernel_spmd

    in_maps, T = _prep_inputs(mode=mode, w16=W16, **inputs)
    nc = _get_nc(T=T, mode=mode)
    res = run_bass_kernel_spmd(nc, in_maps, list(range(NCORES)),
                               trace=trace, **(trace_kwargs or {}))

    out = np.empty((BATCH, T, 2 * HIDDEN), np.float32)
    for c in range(NCORES):
        d, s = c // NSHARD, c % NSHARD
        yt = np.asarray(res.results[c]["yT"], dtype=np.float32)  # [T, H, Bc]
        arr = yt.transpose(2, 0, 1)                              # [Bc, T, H]
        if d == 1:
            arr = arr[:, ::-1]
        out[s * BC:(s + 1) * BC, :, d * HIDDEN:(d + 1) * HIDDEN] = arr
    return out, res


def kernel(**inputs):
    out, _ = _run(inputs)
    return out


# revision 27
# speedup vs baseline: 1.0007x; 1.0007x over previous
"""Bidirectional LSTM encoder (nn_BiEncode) as a Bass/Tile kernel on 8 trn2 cores.

Sharding: direction-split x batch-split. Cores 0-3 run the LEFT (forward-time)
direction on batch shards 0-3 (512 rows each); cores 4-7 run the RIGHT
direction (time-reversed input, handled host-side) on the same batch shards.
Every core runs the identical SPMD program; direction differences live
entirely in the data it is fed (weights + time-reversed x).

Device layout: everything is kept "transposed" (feature dim on partitions,
batch on the free dim) so the scan needs no on-chip transposes:
  x fed as xT[t, i, b], weights as W^T, h/c as [H, B] tiles, output written
  as yT[t, h, b] and un-transposed on the host.

Per timestep the full gate pre-activation g^T[4H, B] is computed as 12
PSUM-accumulated matmuls per 128-row gate tile (8 k-tiles of x-projection +
4 k-tiles of the recurrent term) -- the input projection is fused into the
scan, so no pre-activation tensor is ever materialized. ACT applies
sigmoid/tanh straight out of PSUM; DVE does the cell update.

Dtype choice (all measured on hardware): every candidate runs the PE at
1 column/cycle nominal, but the steady-state matmul issue cadence differs:
fp16 215.8ns vs f32r 226.7ns vs bf16 259ns per [128k x 128m x 512n] matmul
(f32r appears to hit a ~97% power-throttle duty limit that fp16 avoids).
fp16 operands keep a 10-bit mantissa (tf32-class, rel err ~1.3e-3 end to
end) and halve all DMA traffic, so x/w/h/y are all fp16 with fp32 PSUM
accumulation and an fp32 cell state. fp8 was evaluated and rejected:
e4m3 quantization alone gives 1.4e-1 relative error (vs the 2e-2 gate),
and DoubleRow fp8 measures 1.0 cyc/col effective (2x MACs per instr),
so the accuracy-viable 3-term residual scheme would be 1.5x SLOWER.

The kernel is PE-bound at the flat 215.8ns cadence (MFU ~95%); the only
slack is the DMA-bound startup ramp, so the startup transfers are split
across three DMA queues with fine-grained first chunks (see below).
"""

import os

import numpy as np

FRAME_LENGTH = 26
HIDDEN = 512
INPUT = 1024
BATCH = 2048

NCORES = 8
NSHARD = 4                 # batch shards per direction group
BC = BATCH // NSHARD       # 512 batch rows per core

P = 128
KI = INPUT // P            # 8  k-tiles for the input projection
KH = HIDDEN // P           # 4  k-tiles for the recurrent matmul
NJ = HIDDEN // P           # 4  hidden chunks
NM = 4 * HIDDEN // P       # 16 gate m-tiles

# "fp16": fp16 storage+PE operands (half DMA/SBUF, fp32 PSUM accumulation,
#         10-bit mantissa ~ tf32 class, and ~5% faster PE cadence than f32r --
#         fp16 matmuls run below the power-throttle duty limit f32r hits)
# "f32r": fp32 storage, PE in float32r (full-rate at N>=256, ~tf32 precision)
# "bf16": bf16 storage+PE (half DMA/SBUF), fp32 PSUM accumulation
MM_MODE = os.environ.get("BASS_LSTM_MM", "fp16")
W16 = os.environ.get("BASS_LSTM_W16", "1") == "1"   # ship weights fp16
Y16 = os.environ.get("BASS_LSTM_Y16", "1") == "1"   # output y as fp16

_CACHE = {}


def _build(T, Bc, mode, w16, y16):
    import concourse.mybir as mybir
    import concourse.tile as tile
    from concourse import bacc

    dt = mybir.dt
    AF = mybir.ActivationFunctionType

    # matmul-operand storage dtype; the BIR verifier requires fp32r matmul
    # inputs to be produced as fp32r, so x/w/h carry it end-to-end
    io_dt = {"bf16": dt.bfloat16, "fp16": dt.float16}.get(mode, dt.float32r)
    w16 = w16 and mode == "f32r"
    w_dt = dt.float16 if w16 else io_dt
    y_dt = dt.float16 if y16 else io_dt
    y_copy = y_dt != io_dt

    nc = bacc.Bacc("TRN2", target_bir_lowering=False, debug=False,
                   num_devices=NCORES)
    # All inputs are host-laid-out to match the SBUF tiles exactly, so every
    # DMA moves maximally contiguous runs per partition (large descriptors:
    # x 16KB/partition per step, w_ih 8KB/partition per j-chunk). 1-2KB
    # descriptors measurably halve per-queue DMA throughput.
    xT = nc.dram_tensor("xT", [T, P, KI, Bc], io_dt,
                        kind="ExternalInput").ap()
    w_ih = nc.dram_tensor("w_ih", [P, NJ, KI, 4 * P], w_dt,
                          kind="ExternalInput").ap()
    w_hh = nc.dram_tensor("w_hh", [P, NJ, KH, 4 * P], w_dt,
                          kind="ExternalInput").ap()
    bias = nc.dram_tensor("bias", [P, NM], dt.float32, kind="ExternalInput").ap()
    yT = nc.dram_tensor("yT", [T, HIDDEN, Bc], y_dt, kind="ExternalOutput").ap()

    with tile.TileContext(nc) as tc:
        with tc.tile_pool(name="wpool", bufs=1) as wp, \
             tc.tile_pool(name="xpool", bufs=2) as xp, \
             tc.tile_pool(name="work", bufs=1) as wk, \
             tc.tile_pool(name="psum", bufs=2, space="PSUM") as pp:

            # Startup-latency plan (the kernel is PE-bound; all slack is in
            # the DMA-bound ramp): DMA trigger issue costs ~0.64us each on an
            # engine sequencer, so the startup transfers are spread across
            # four engines' HWDGE queues to issue in parallel:
            #   sync:   w_ih fp16 (j0 per-k slices first, then j1..j3)
            #   scalar: w_hh fp16
            #   gpsimd: t=0 x slices + t=1 x prefetch
            # (DVE does the fp16->f32r upcasts but cannot issue DMAs)
            # Subtile dependency tracking lets the first matmul start once
            # x[k=0] and the k=0 slice of the j0 weight upcast land.
            # bias rides the gpsimd queue: keeps the sync queue's first
            # trigger for the j0 weights; lands ~11us, first ACT needs it
            # ~16.5us
            bias_sb = wp.tile([P, NM], dt.float32, tag="bias")
            nc.gpsimd.dma_start(out=bias_sb, in_=bias[:, :])

            def upcast(dst_f32r, src_f16):
                # fp16 -> f32r convert (f32r-typed output keeps the BIR
                # verifier happy about fp32r matmul operands)
                nc.vector.tensor_copy(dst_f32r, src_f16)

            # t=0 x slices ride the scalar queue (ahead of w_hh, which isn't
            # needed until t=1) so they land while the sync queue streams j0
            # weights; the gpsimd SWDGE queue spins up ~2.5us later and only
            # carries the t=1 prefetch.
            xt0 = xp.tile([P, KI, Bc], io_dt, tag="x")
            for k in range(KI):
                nc.scalar.dma_start(out=xt0[:, k, :], in_=xT[0, :, k, :])

            w_ih_sb = []          # [j] -> [P, KI, 4P] f32r tile
            w_hh_sb = []          # [j] -> [P, KH, 4P] f32r tile
            if w16:
                # j=0 flows per-k for minimum first-matmul latency
                wt0 = wp.tile([P, KI, 4 * P], io_dt, tag="wih0")
                for k in range(KI):
                    wfk = wp.tile([P, 4 * P], dt.float16, tag="wsk", bufs=4)
                    nc.sync.dma_start(out=wfk, in_=w_ih[:, 0, k, :])
                    upcast(wt0[:, k, :], wfk)
                w_ih_sb.append(wt0)
                for j in range(1, NJ):
                    wf = wp.tile([P, KI, 4 * P], dt.float16, tag="ws", bufs=2)
                    nc.sync.dma_start(out=wf, in_=w_ih[:, j, :, :])
                    wt = wp.tile([P, KI, 4 * P], io_dt, tag=f"wih{j}")
                    upcast(wt, wf)
                    w_ih_sb.append(wt)
                for j in range(NJ):
                    wf = wp.tile([P, KH, 4 * P], dt.float16, tag="whs", bufs=2)
                    nc.scalar.dma_start(out=wf, in_=w_hh[:, j, :, :])
                    wt = wp.tile([P, KH, 4 * P], io_dt, tag=f"whh{j}")
                    upcast(wt, wf)
                    w_hh_sb.append(wt)
            else:
                # j0 per-k (first matmul waits only 128KB), j1-3 as
                # half-chunks (4KB-contiguous descriptors, finer pipelining).
                # j1 rides the scalar queue right behind the t=0 x slices so
                # it lands before the PE finishes chewing j0; j2/j3 follow j0
                # on the sync queue; w_hh (not needed until t=1, ~50us in)
                # queues last on scalar.
                h2 = KI // 2
                for j in range(NJ):
                    wt = wp.tile([P, KI, 4 * P], io_dt, tag=f"wihf{j}")
                    if j == 0:
                        for k in range(KI):
                            nc.sync.dma_start(out=wt[:, k, :],
                                              in_=w_ih[:, 0, k, :])
                    else:
                        eng = nc.scalar if j == 1 else nc.sync
                        eng.dma_start(out=wt[:, :h2, :],
                                      in_=w_ih[:, j, :h2, :])
                        eng.dma_start(out=wt[:, h2:, :],
                                      in_=w_ih[:, j, h2:, :])
                    w_ih_sb.append(wt)
                for j in range(NJ):
                    wt = wp.tile([P, KH, 4 * P], io_dt, tag=f"whh{j}")
                    nc.scalar.dma_start(out=wt, in_=w_hh[:, j, :, :])
                    w_hh_sb.append(wt)
            # prefetch t=1's x on the gpsimd engine's queue
            xt1 = None
            if T > 1:
                xt1 = xp.tile([P, KI, Bc], io_dt, tag="x")
                nc.gpsimd.dma_start(out=xt1, in_=xT[1])

            # h0 = c0 = 0, so step 0 skips the recurrent matmuls and the
            # f*c term entirely -- no initial state tiles needed (memset
            # can't produce float32r anyway).
            h_cur, c_cur = [], []

            GATE_FUNCS = (AF.Sigmoid, AF.Sigmoid, AF.Tanh, AF.Sigmoid)

            for t in range(T):
                if t == 0:
                    xt = xt0
                elif t == 1:
                    xt = xt1
                else:
                    xt = xp.tile([P, KI, Bc], io_dt, tag="x")
                    nc.sync.dma_start(out=xt, in_=xT[t])

                h_next, c_next = [], []
                for j in range(NJ):
                    acts = {}

                    def gate_chain(gi):
                        m = gi * NJ + j
                        ps = pp.tile([P, Bc], dt.float32, tag=f"ps{gi}",
                                     name=f"ps_{t}_{j}_{gi}")
                        for k in range(KI):
                            nc.tensor.matmul(
                                ps, lhsT=w_ih_sb[j][:, k, gi * P:(gi + 1) * P],
                                rhs=xt[:, k, :],
                                start=(k == 0),
                                stop=(t == 0 and k == KI - 1))
                        if t > 0:
                            for k in range(KH):
                                nc.tensor.matmul(
                                    ps, lhsT=w_hh_sb[j][:, k, gi * P:(gi + 1) * P],
                                    rhs=h_cur[k],
                                    start=False, stop=(k == KH - 1))
                        gt = wk.tile([P, Bc], dt.float32, tag=f"g{gi}",
                                     bufs=2, name=f"gt_{t}_{j}_{gi}")
                        nc.scalar.activation(gt, ps, GATE_FUNCS[gi],
                                             bias=bias_sb[:, m:m + 1])
                        acts[gi] = gt

                    # t=0: c0=0 makes the f-gate's contribution f*c0 vanish,
                    # so its whole matmul chain + sigmoid are skipped.
                    # The o-gate chain is emitted AFTER the cell update and
                    # tanh(c): the PE stream is unchanged, but tanh(c) sits
                    # ahead of sigmoid(o) in the ACT queue, shortening the
                    # critical h chain after the final matmul of the run.
                    for gi in ((0, 2) if t == 0 else (0, 1, 2)):
                        gate_chain(gi)
                    i_t, g_t = acts[0], acts[2]
                    cn = wk.tile([P, Bc], dt.float32, tag=f"c{j}")
                    if t == 0:
                        nc.vector.tensor_mul(cn, i_t, g_t)
                    else:
                        f_t = acts[1]
                        u = wk.tile([P, Bc], dt.float32, tag="u")
                        nc.vector.tensor_mul(u, i_t, g_t)
                        v = wk.tile([P, Bc], dt.float32, tag="v")
                        nc.vector.tensor_mul(v, f_t, c_cur[j])
                        nc.vector.tensor_add(cn, u, v)
                    th = wk.tile([P, Bc], dt.float32, tag="th")
                    nc.scalar.activation(th, cn, AF.Tanh)
                    gate_chain(3)
                    o_t = acts[3]
                    hn = wk.tile([P, Bc], io_dt, tag=f"h{j}", bufs=2)
                    nc.vector.tensor_mul(hn, o_t, th)
                    if y_copy:
                        yh = wk.tile([P, Bc], dt.float16, tag="yh")
                        nc.vector.tensor_copy(yh, hn)
                        nc.sync.dma_start(out=yT[t, j * P:(j + 1) * P, :],
                                          in_=yh)
                    else:
                        nc.sync.dma_start(out=yT[t, j * P:(j + 1) * P, :],
                                          in_=hn)
                    h_next.append(hn)
                    c_next.append(cn)
                h_cur, c_cur = h_next, c_next

    nc.compile()
    return nc


def _get_nc(T=FRAME_LENGTH, Bc=BC, mode=MM_MODE, w16=W16, y16=Y16):
    key = (T, Bc, mode, w16, y16)
    if key not in _CACHE:
        _CACHE[key] = _build(T, Bc, mode, w16, y16)
    return _CACHE[key]


def _prep_inputs(embed_feats, w_ih_l, w_hh_l, b_ih_l, b_hh_l,
                 w_ih_r, w_hh_r, b_ih_r, b_hh_r, mode, w16):
    import ml_dtypes

    io_np = {"bf16": ml_dtypes.bfloat16,
             "fp16": np.float16}.get(mode, np.float32)
    w_np = np.float16 if (w16 and mode == "f32r") else io_np
    T = embed_feats.shape[1]

    w = {
        0: (np.asarray(w_ih_l), np.asarray(w_hh_l),
            np.asarray(b_ih_l) + np.asarray(b_hh_l)),
        1: (np.asarray(w_ih_r), np.asarray(w_hh_r),
            np.asarray(b_ih_r) + np.asarray(b_hh_r)),
    }
    x = np.asarray(embed_feats)

    # j-major column permutation of the 4H gate dim: block j holds the four
    # gates' columns for hidden chunk j, so each j-chunk loads contiguously
    j_idx, g_idx, c_idx = np.meshgrid(
        np.arange(NJ), np.arange(4), np.arange(P), indexing="ij")
    perm = (g_idx * (NJ * P) + j_idx * P + c_idx).reshape(-1)

    def dev_w(wmat, kt):
        # [IN, 4H] -> j-major column perm -> [P, NJ, kt, 4P] (the on-chip
        # tile layout, so each j-chunk DMA is fully contiguous per partition)
        wT = wmat.T[:, perm]                      # [IN, 4H] j-major cols
        wr = wT.reshape(kt, P, NJ, 4 * P)         # (k p) rows
        return np.ascontiguousarray(wr.transpose(1, 2, 0, 3)).astype(w_np)

    in_maps = []
    for c in range(NCORES):
        d, s = c // NSHARD, c % NSHARD
        xs = x[s * BC:(s + 1) * BC]
        if d == 1:
            xs = xs[:, ::-1]
        # [Bc, T, IN] -> [T, P, KI, Bc] (on-chip step-tile layout)
        xT = np.ascontiguousarray(
            xs.transpose(1, 2, 0).reshape(T, KI, P, BC)
            .transpose(0, 2, 1, 3)).astype(io_np)
        bias = np.ascontiguousarray(
            w[d][2].astype(np.float32).reshape(NM, P).T)
        in_maps.append({"xT": xT, "w_ih": dev_w(w[d][0], KI),
                        "w_hh": dev_w(w[d][1], KH), "bias": bias})
    return in_maps, T


def _run(inputs, mode=MM_MODE, trace=False, trace_kwargs=None):
    from concourse.bass_utils import run_bass_kernel_spmd

    in_maps, T = _prep_inputs(mode=mode, w16=W16, **inputs)
    nc = _get_nc(T=T, mode=mode)
    res = run_bass_kernel_spmd(nc, in_maps, list(range(NCORES)),
                               trace=trace, **(trace_kwargs or {}))

    out = np.empty((BATCH, T, 2 * HIDDEN), np.float32)
    for c in range(NCORES):
        d, s = c // NSHARD, c % NSHARD
        yt = np.asarray(res.results[c]["yT"], dtype=np.float32)  # [T, H, Bc]
        arr = yt.transpose(2, 0, 1)                              # [Bc, T, H]
        if d == 1:
            arr = arr[:, ::-1]
        out[s * BC:(s + 1) * BC, :, d * HIDDEN:(d + 1) * HIDDEN] = arr
    return out, res


def kernel(**inputs):
    out, _ = _run(inputs)
    return out


# revision 28
# speedup vs baseline: 1.0026x; 1.0018x over previous
"""Bidirectional LSTM encoder (nn_BiEncode) as a Bass/Tile kernel on 8 trn2 cores.

Sharding: direction-split x batch-split. Cores 0-3 run the LEFT (forward-time)
direction on batch shards 0-3 (512 rows each); cores 4-7 run the RIGHT
direction (time-reversed input, handled host-side) on the same batch shards.
Every core runs the identical SPMD program; direction differences live
entirely in the data it is fed (weights + time-reversed x).

Device layout: everything is kept "transposed" (feature dim on partitions,
batch on the free dim) so the scan needs no on-chip transposes:
  x fed as xT[t, i, b], weights as W^T, h/c as [H, B] tiles, output written
  as yT[t, h, b] and un-transposed on the host.

Per timestep the full gate pre-activation g^T[4H, B] is computed as 12
PSUM-accumulated matmuls per 128-row gate tile (8 k-tiles of x-projection +
4 k-tiles of the recurrent term) -- the input projection is fused into the
scan, so no pre-activation tensor is ever materialized. ACT applies
sigmoid/tanh straight out of PSUM; DVE does the cell update.

Dtype choice (all measured on hardware): every candidate runs the PE at
1 column/cycle nominal, but the steady-state matmul issue cadence differs:
fp16 215.8ns vs f32r 226.7ns vs bf16 259ns per [128k x 128m x 512n] matmul
(f32r appears to hit a ~97% power-throttle duty limit that fp16 avoids).
fp16 operands keep a 10-bit mantissa (tf32-class, rel err ~1.3e-3 end to
end) and halve all DMA traffic, so x/w/h/y are all fp16 with fp32 PSUM
accumulation and an fp32 cell state. fp8 was evaluated and rejected:
e4m3 quantization alone gives 1.4e-1 relative error (vs the 2e-2 gate),
and DoubleRow fp8 measures 1.0 cyc/col effective (2x MACs per instr),
so the accuracy-viable 3-term residual scheme would be 1.5x SLOWER.

The kernel is PE-bound at the flat 215.8ns cadence (MFU ~95%); the only
slack is the DMA-bound startup ramp, so the startup transfers are split
across three DMA queues with fine-grained first chunks (see below).
"""

import os

import numpy as np

FRAME_LENGTH = 26
HIDDEN = 512
INPUT = 1024
BATCH = 2048

NCORES = 8
NSHARD = 4                 # batch shards per direction group
BC = BATCH // NSHARD       # 512 batch rows per core

P = 128
KI = INPUT // P            # 8  k-tiles for the input projection
KH = HIDDEN // P           # 4  k-tiles for the recurrent matmul
NJ = HIDDEN // P           # 4  hidden chunks
NM = 4 * HIDDEN // P       # 16 gate m-tiles

# "fp16": fp16 storage+PE operands (half DMA/SBUF, fp32 PSUM accumulation,
#         10-bit mantissa ~ tf32 class, and ~5% faster PE cadence than f32r --
#         fp16 matmuls run below the power-throttle duty limit f32r hits)
# "f32r": fp32 storage, PE in float32r (full-rate at N>=256, ~tf32 precision)
# "bf16": bf16 storage+PE (half DMA/SBUF), fp32 PSUM accumulation
MM_MODE = os.environ.get("BASS_LSTM_MM", "fp16")
W16 = os.environ.get("BASS_LSTM_W16", "1") == "1"   # ship weights fp16
Y16 = os.environ.get("BASS_LSTM_Y16", "1") == "1"   # output y as fp16

_CACHE = {}


def _build(T, Bc, mode, w16, y16):
    import concourse.mybir as mybir
    import concourse.tile as tile
    from concourse import bacc

    dt = mybir.dt
    AF = mybir.ActivationFunctionType

    # matmul-operand storage dtype; the BIR verifier requires fp32r matmul
    # inputs to be produced as fp32r, so x/w/h carry it end-to-end
    io_dt = {"bf16": dt.bfloat16, "fp16": dt.float16}.get(mode, dt.float32r)
    w16 = w16 and mode == "f32r"
    w_dt = dt.float16 if w16 else io_dt
    y_dt = dt.float16 if y16 else io_dt
    y_copy = y_dt != io_dt

    nc = bacc.Bacc("TRN2", target_bir_lowering=False, debug=False,
                   num_devices=NCORES)
    # All inputs are host-laid-out to match the SBUF tiles exactly, so every
    # DMA moves maximally contiguous runs per partition (large descriptors:
    # x 16KB/partition per step, w_ih 8KB/partition per j-chunk). 1-2KB
    # descriptors measurably halve per-queue DMA throughput.
    xT = nc.dram_tensor("xT", [T, P, KI, Bc], io_dt,
                        kind="ExternalInput").ap()
    w_ih = nc.dram_tensor("w_ih", [P, NJ, KI, 4 * P], w_dt,
                          kind="ExternalInput").ap()
    w_hh = nc.dram_tensor("w_hh", [P, NJ, KH, 4 * P], w_dt,
                          kind="ExternalInput").ap()
    bias = nc.dram_tensor("bias", [P, NM], dt.float32, kind="ExternalInput").ap()
    yT = nc.dram_tensor("yT", [T, HIDDEN, Bc], y_dt, kind="ExternalOutput").ap()

    with tile.TileContext(nc) as tc:
        with tc.tile_pool(name="wpool", bufs=1) as wp, \
             tc.tile_pool(name="xpool", bufs=2) as xp, \
             tc.tile_pool(name="work", bufs=1) as wk, \
             tc.tile_pool(name="psum", bufs=2, space="PSUM") as pp:

            # Startup-latency plan (the kernel is PE-bound; all slack is in
            # the DMA-bound ramp): DMA trigger issue costs ~0.64us each on an
            # engine sequencer, so the startup transfers are spread across
            # four engines' HWDGE queues to issue in parallel:
            #   sync:   w_ih fp16 (j0 per-k slices first, then j1..j3)
            #   scalar: w_hh fp16
            #   gpsimd: t=0 x slices + t=1 x prefetch
            # (DVE does the fp16->f32r upcasts but cannot issue DMAs)
            # Subtile dependency tracking lets the first matmul start once
            # x[k=0] and the k=0 slice of the j0 weight upcast land.
            # bias rides the gpsimd queue: keeps the sync queue's first
            # trigger for the j0 weights; lands ~11us, first ACT needs it
            # ~16.5us
            bias_sb = wp.tile([P, NM], dt.float32, tag="bias")
            nc.gpsimd.dma_start(out=bias_sb, in_=bias[:, :])

            def upcast(dst_f32r, src_f16):
                # fp16 -> f32r convert (f32r-typed output keeps the BIR
                # verifier happy about fp32r matmul operands)
                nc.vector.tensor_copy(dst_f32r, src_f16)

            # t=0 x slices ride the scalar queue (ahead of w_hh, which isn't
            # needed until t=1) so they land while the sync queue streams j0
            # weights; the gpsimd SWDGE queue spins up ~2.5us later and only
            # carries the t=1 prefetch.
            xt0 = xp.tile([P, KI, Bc], io_dt, tag="x")
            for k in range(KI):
                nc.scalar.dma_start(out=xt0[:, k, :], in_=xT[0, :, k, :])

            w_ih_sb = []          # [j] -> [P, KI, 4P] f32r tile
            w_hh_sb = []          # [j] -> [P, KH, 4P] f32r tile
            if w16:
                # j=0 flows per-k for minimum first-matmul latency
                wt0 = wp.tile([P, KI, 4 * P], io_dt, tag="wih0")
                for k in range(KI):
                    wfk = wp.tile([P, 4 * P], dt.float16, tag="wsk", bufs=4)
                    nc.sync.dma_start(out=wfk, in_=w_ih[:, 0, k, :])
                    upcast(wt0[:, k, :], wfk)
                w_ih_sb.append(wt0)
                for j in range(1, NJ):
                    wf = wp.tile([P, KI, 4 * P], dt.float16, tag="ws", bufs=2)
                    nc.sync.dma_start(out=wf, in_=w_ih[:, j, :, :])
                    wt = wp.tile([P, KI, 4 * P], io_dt, tag=f"wih{j}")
                    upcast(wt, wf)
                    w_ih_sb.append(wt)
                for j in range(NJ):
                    wf = wp.tile([P, KH, 4 * P], dt.float16, tag="whs", bufs=2)
                    nc.scalar.dma_start(out=wf, in_=w_hh[:, j, :, :])
                    wt = wp.tile([P, KH, 4 * P], io_dt, tag=f"whh{j}")
                    upcast(wt, wf)
                    w_hh_sb.append(wt)
            else:
                # j0 per-k (first matmul waits only 128KB), j1-3 as
                # half-chunks (4KB-contiguous descriptors, finer pipelining).
                # j1 rides the scalar queue right behind the t=0 x slices so
                # it lands before the PE finishes chewing j0; j2/j3 follow j0
                # on the sync queue; w_hh (not needed until t=1, ~50us in)
                # queues last on scalar.
                h2 = KI // 2
                for j in range(NJ):
                    wt = wp.tile([P, KI, 4 * P], io_dt, tag=f"wihf{j}")
                    if j == 0:
                        for k in range(KI):
                            nc.sync.dma_start(out=wt[:, k, :],
                                              in_=w_ih[:, 0, k, :])
                    else:
                        eng = nc.scalar if j == 1 else nc.sync
                        eng.dma_start(out=wt[:, :h2, :],
                                      in_=w_ih[:, j, :h2, :])
                        eng.dma_start(out=wt[:, h2:, :],
                                      in_=w_ih[:, j, h2:, :])
                    w_ih_sb.append(wt)
                for j in range(NJ):
                    wt = wp.tile([P, KH, 4 * P], io_dt, tag=f"whh{j}")
                    nc.scalar.dma_start(out=wt, in_=w_hh[:, j, :, :])
                    w_hh_sb.append(wt)
            # prefetch t=1's x on the gpsimd engine's queue
            xt1 = None
            if T > 1:
                xt1 = xp.tile([P, KI, Bc], io_dt, tag="x")
                nc.gpsimd.dma_start(out=xt1, in_=xT[1])

            # h0 = c0 = 0, so step 0 skips the recurrent matmuls and the
            # f*c term entirely -- no initial state tiles needed (memset
            # can't produce float32r anyway).
            h_cur, c_cur = [], []

            GATE_FUNCS = (AF.Sigmoid, AF.Sigmoid, AF.Tanh, AF.Sigmoid)

            for t in range(T):
                if t == 0:
                    xt = xt0
                elif t == 1:
                    xt = xt1
                else:
                    xt = xp.tile([P, KI, Bc], io_dt, tag="x")
                    nc.sync.dma_start(out=xt, in_=xT[t])

                h_next, c_next = [], []
                for j in range(NJ):
                    acts = {}
                    # t=0: c0=0 makes the f-gate's contribution f*c0 vanish,
                    # so its whole matmul chain + sigmoid are skipped
                    for gi in ((0, 2, 3) if t == 0 else range(4)):
                        m = gi * NJ + j
                        ps = pp.tile([P, Bc], dt.float32, tag=f"ps{gi}")
                        for k in range(KI):
                            nc.tensor.matmul(
                                ps, lhsT=w_ih_sb[j][:, k, gi * P:(gi + 1) * P],
                                rhs=xt[:, k, :],
                                start=(k == 0),
                                stop=(t == 0 and k == KI - 1))
                        if t > 0:
                            for k in range(KH):
                                nc.tensor.matmul(
                                    ps, lhsT=w_hh_sb[j][:, k, gi * P:(gi + 1) * P],
                                    rhs=h_cur[k],
                                    start=False, stop=(k == KH - 1))
                        gt = wk.tile([P, Bc], dt.float32, tag=f"g{gi}",
                                     bufs=2)
                        nc.scalar.activation(gt, ps, GATE_FUNCS[gi],
                                             bias=bias_sb[:, m:m + 1])
                        acts[gi] = gt
                    i_t, g_t, o_t = acts[0], acts[2], acts[3]
                    cn = wk.tile([P, Bc], dt.float32, tag=f"c{j}")
                    if t == 0:
                        nc.vector.tensor_mul(cn, i_t, g_t)
                    else:
                        f_t = acts[1]
                        u = wk.tile([P, Bc], dt.float32, tag="u")
                        nc.vector.tensor_mul(u, i_t, g_t)
                        v = wk.tile([P, Bc], dt.float32, tag="v")
                        nc.vector.tensor_mul(v, f_t, c_cur[j])
                        nc.vector.tensor_add(cn, u, v)
                    th = wk.tile([P, Bc], dt.float32, tag="th")
                    nc.scalar.activation(th, cn, AF.Tanh)
                    hn = wk.tile([P, Bc], io_dt, tag=f"h{j}", bufs=2)
                    nc.vector.tensor_mul(hn, o_t, th)
                    if y_copy:
                        yh = wk.tile([P, Bc], dt.float16, tag="yh")
                        nc.vector.tensor_copy(yh, hn)
                        nc.sync.dma_start(out=yT[t, j * P:(j + 1) * P, :],
                                          in_=yh)
                    else:
                        nc.sync.dma_start(out=yT[t, j * P:(j + 1) * P, :],
                                          in_=hn)
                    h_next.append(hn)
                    c_next.append(cn)
                h_cur, c_cur = h_next, c_next

    nc.compile()
    return nc


def _get_nc(T=FRAME_LENGTH, Bc=BC, mode=MM_MODE, w16=W16, y16=Y16):
    key = (T, Bc, mode, w16, y16)
    if key not in _CACHE:
        _CACHE[key] = _build(T, Bc, mode, w16, y16)
    return _CACHE[key]


def _prep_inputs(embed_feats, w_ih_l, w_hh_l, b_ih_l, b_hh_l,
                 w_ih_r, w_hh_r, b_ih_r, b_hh_r, mode, w16):
    import ml_dtypes

    io_np = {"bf16": ml_dtypes.bfloat16,
             "fp16": np.float16}.get(mode, np.float32)
    w_np = np.float16 if (w16 and mode == "f32r") else io_np
    T = embed_feats.shape[1]

    w = {
        0: (np.asarray(w_ih_l), np.asarray(w_hh_l),
            np.asarray(b_ih_l) + np.asarray(b_hh_l)),
        1: (np.asarray(w_ih_r), np.asarray(w_hh_r),
            np.asarray(b_ih_r) + np.asarray(b_hh_r)),
    }
    x = np.asarray(embed_feats)

    # j-major column permutation of the 4H gate dim: block j holds the four
    # gates' columns for hidden chunk j, so each j-chunk loads contiguously
    j_idx, g_idx, c_idx = np.meshgrid(
        np.arange(NJ), np.arange(4), np.arange(P), indexing="ij")
    perm = (g_idx * (NJ * P) + j_idx * P + c_idx).reshape(-1)

    def dev_w(wmat, kt):
        # [IN, 4H] -> j-major column perm -> [P, NJ, kt, 4P] (the on-chip
        # tile layout, so each j-chunk DMA is fully contiguous per partition)
        wT = wmat.T[:, perm]                      # [IN, 4H] j-major cols
        wr = wT.reshape(kt, P, NJ, 4 * P)         # (k p) rows
        return np.ascontiguousarray(wr.transpose(1, 2, 0, 3)).astype(w_np)

    in_maps = []
    for c in range(NCORES):
        d, s = c // NSHARD, c % NSHARD
        xs = x[s * BC:(s + 1) * BC]
        if d == 1:
            xs = xs[:, ::-1]
        # [Bc, T, IN] -> [T, P, KI, Bc] (on-chip step-tile layout)
        xT = np.ascontiguousarray(
            xs.transpose(1, 2, 0).reshape(T, KI, P, BC)
            .transpose(0, 2, 1, 3)).astype(io_np)
        bias = np.ascontiguousarray(
            w[d][2].astype(np.float32).reshape(NM, P).T)
        in_maps.append({"xT": xT, "w_ih": dev_w(w[d][0], KI),
                        "w_hh": dev_w(w[d][1], KH), "bias": bias})
    return in_maps, T


def _run(inputs, mode=MM_MODE, trace=False, trace_kwargs=None):
    from concourse.bass_utils import run_bass_kernel_spmd

    in_maps, T = _prep_inputs(mode=mode, w16=W16, **inputs)
    nc = _get_nc(T=T, mode=mode)
    res = run_bass_kernel_spmd(nc, in_maps, list(range(NCORES)),
                               trace=trace, **(trace_kwargs or {}))

    out = np.empty((BATCH, T, 2 * HIDDEN), np.float32)
    for c in range(NCORES):
        d, s = c // NSHARD, c % NSHARD
        yt = np.asarray(res.results[c]["yT"], dtype=np.float32)  # [T, H, Bc]
        arr = yt.transpose(2, 0, 1)                              # [Bc, T, H]
        if d == 1:
            arr = arr[:, ::-1]
        out[s * BC:(s + 1) * BC, :, d * HIDDEN:(d + 1) * HIDDEN] = arr
    return out, res


def kernel(**inputs):
    out, _ = _run(inputs)
    return out


# revision 29
# speedup vs baseline: 1.0037x; 1.0012x over previous
"""Bidirectional LSTM encoder (nn_BiEncode) as a Bass/Tile kernel on 8 trn2 cores.

Sharding: direction-split x batch-split. Cores 0-3 run the LEFT (forward-time)
direction on batch shards 0-3 (512 rows each); cores 4-7 run the RIGHT
direction (time-reversed input, handled host-side) on the same batch shards.
Every core runs the identical SPMD program; direction differences live
entirely in the data it is fed (weights + time-reversed x).

Device layout: everything is kept "transposed" (feature dim on partitions,
batch on the free dim) so the scan needs no on-chip transposes:
  x fed as xT[t, i, b], weights as W^T, h/c as [H, B] tiles, output written
  as yT[t, h, b] and un-transposed on the host.

Per timestep the full gate pre-activation g^T[4H, B] is computed as 12
PSUM-accumulated matmuls per 128-row gate tile (8 k-tiles of x-projection +
4 k-tiles of the recurrent term) -- the input projection is fused into the
scan, so no pre-activation tensor is ever materialized. ACT applies
sigmoid/tanh straight out of PSUM; DVE does the cell update.

Dtype choice (all measured on hardware): every candidate runs the PE at
1 column/cycle nominal, but the steady-state matmul issue cadence differs:
fp16 215.8ns vs f32r 226.7ns vs bf16 259ns per [128k x 128m x 512n] matmul
(f32r appears to hit a ~97% power-throttle duty limit that fp16 avoids).
fp16 operands keep a 10-bit mantissa (tf32-class, rel err ~1.3e-3 end to
end) and halve all DMA traffic, so x/w/h/y are all fp16 with fp32 PSUM
accumulation and an fp32 cell state. fp8 was evaluated and rejected:
e4m3 quantization alone gives 1.4e-1 relative error (vs the 2e-2 gate),
and DoubleRow fp8 measures 1.0 cyc/col effective (2x MACs per instr),
so the accuracy-viable 3-term residual scheme would be 1.5x SLOWER.

The kernel is PE-bound at the flat 215.8ns cadence (MFU ~95%); the only
slack is the DMA-bound startup ramp, so the startup transfers are split
across three DMA queues with fine-grained first chunks (see below).
"""

import os

import numpy as np

FRAME_LENGTH = 26
HIDDEN = 512
INPUT = 1024
BATCH = 2048

NCORES = 8
NSHARD = 4                 # batch shards per direction group
BC = BATCH // NSHARD       # 512 batch rows per core

P = 128
KI = INPUT // P            # 8  k-tiles for the input projection
KH = HIDDEN // P           # 4  k-tiles for the recurrent matmul
NJ = HIDDEN // P           # 4  hidden chunks
NM = 4 * HIDDEN // P       # 16 gate m-tiles

# "fp16": fp16 storage+PE operands (half DMA/SBUF, fp32 PSUM accumulation,
#         10-bit mantissa ~ tf32 class, and ~5% faster PE cadence than f32r --
#         fp16 matmuls run below the power-throttle duty limit f32r hits)
# "f32r": fp32 storage, PE in float32r (full-rate at N>=256, ~tf32 precision)
# "bf16": bf16 storage+PE (half DMA/SBUF), fp32 PSUM accumulation
MM_MODE = os.environ.get("BASS_LSTM_MM", "fp16")
W16 = os.environ.get("BASS_LSTM_W16", "1") == "1"   # ship weights fp16
Y16 = os.environ.get("BASS_LSTM_Y16", "1") == "1"   # output y as fp16

_CACHE = {}


def _build(T, Bc, mode, w16, y16):
    import concourse.mybir as mybir
    import concourse.tile as tile
    from concourse import bacc

    dt = mybir.dt
    AF = mybir.ActivationFunctionType

    # matmul-operand storage dtype; the BIR verifier requires fp32r matmul
    # inputs to be produced as fp32r, so x/w/h carry it end-to-end
    io_dt = {"bf16": dt.bfloat16, "fp16": dt.float16}.get(mode, dt.float32r)
    w16 = w16 and mode == "f32r"
    w_dt = dt.float16 if w16 else io_dt
    y_dt = dt.float16 if y16 else io_dt
    y_copy = y_dt != io_dt

    nc = bacc.Bacc("TRN2", target_bir_lowering=False, debug=False,
                   num_devices=NCORES)
    # All inputs are host-laid-out to match the SBUF tiles exactly, so every
    # DMA moves maximally contiguous runs per partition (large descriptors:
    # x 16KB/partition per step, w_ih 8KB/partition per j-chunk). 1-2KB
    # descriptors measurably halve per-queue DMA throughput.
    xT = nc.dram_tensor("xT", [T, P, KI, Bc], io_dt,
                        kind="ExternalInput").ap()
    w_ih = nc.dram_tensor("w_ih", [P, NJ, KI, 4 * P], w_dt,
                          kind="ExternalInput").ap()
    w_hh = nc.dram_tensor("w_hh", [P, NJ, KH, 4 * P], w_dt,
                          kind="ExternalInput").ap()
    bias = nc.dram_tensor("bias", [P, NM], dt.float32, kind="ExternalInput").ap()
    yT = nc.dram_tensor("yT", [T, HIDDEN, Bc], y_dt, kind="ExternalOutput").ap()

    with tile.TileContext(nc) as tc:
        with tc.tile_pool(name="wpool", bufs=1) as wp, \
             tc.tile_pool(name="xpool", bufs=2) as xp, \
             tc.tile_pool(name="work", bufs=1) as wk, \
             tc.tile_pool(name="psum", bufs=2, space="PSUM") as pp:

            # Startup-latency plan (the kernel is PE-bound; all slack is in
            # the DMA-bound ramp): DMA trigger issue costs ~0.64us each on an
            # engine sequencer, so the startup transfers are spread across
            # four engines' HWDGE queues to issue in parallel:
            #   sync:   w_ih fp16 (j0 per-k slices first, then j1..j3)
            #   scalar: w_hh fp16
            #   gpsimd: t=0 x slices + t=1 x prefetch
            # (DVE does the fp16->f32r upcasts but cannot issue DMAs)
            # Subtile dependency tracking lets the first matmul start once
            # x[k=0] and the k=0 slice of the j0 weight upcast land.
            # bias rides the gpsimd queue: keeps the sync queue's first
            # trigger for the j0 weights; lands ~11us, first ACT needs it
            # ~16.5us
            bias_sb = wp.tile([P, NM], dt.float32, tag="bias")
            nc.gpsimd.dma_start(out=bias_sb, in_=bias[:, :])

            def upcast(dst_f32r, src_f16):
                # fp16 -> f32r convert (f32r-typed output keeps the BIR
                # verifier happy about fp32r matmul operands)
                nc.vector.tensor_copy(dst_f32r, src_f16)

            # t=0 x slices ride the scalar queue (ahead of w_hh, which isn't
            # needed until t=1) so they land while the sync queue streams j0
            # weights; the gpsimd SWDGE queue spins up ~2.5us later and only
            # carries the t=1 prefetch.
            xt0 = xp.tile([P, KI, Bc], io_dt, tag="x")
            for k in range(KI):
                nc.scalar.dma_start(out=xt0[:, k, :], in_=xT[0, :, k, :])

            w_ih_sb = []          # [j] -> [P, KI, 4P] f32r tile
            w_hh_sb = []          # [j] -> [P, KH, 4P] f32r tile
            if w16:
                # j=0 flows per-k for minimum first-matmul latency
                wt0 = wp.tile([P, KI, 4 * P], io_dt, tag="wih0")
                for k in range(KI):
                    wfk = wp.tile([P, 4 * P], dt.float16, tag="wsk", bufs=4)
                    nc.sync.dma_start(out=wfk, in_=w_ih[:, 0, k, :])
                    upcast(wt0[:, k, :], wfk)
                w_ih_sb.append(wt0)
                for j in range(1, NJ):
                    wf = wp.tile([P, KI, 4 * P], dt.float16, tag="ws", bufs=2)
                    nc.sync.dma_start(out=wf, in_=w_ih[:, j, :, :])
                    wt = wp.tile([P, KI, 4 * P], io_dt, tag=f"wih{j}")
                    upcast(wt, wf)
                    w_ih_sb.append(wt)
                for j in range(NJ):
                    wf = wp.tile([P, KH, 4 * P], dt.float16, tag="whs", bufs=2)
                    nc.scalar.dma_start(out=wf, in_=w_hh[:, j, :, :])
                    wt = wp.tile([P, KH, 4 * P], io_dt, tag=f"whh{j}")
                    upcast(wt, wf)
                    w_hh_sb.append(wt)
            else:
                # j0 per-k (first matmul waits only 128KB), j1-3 as
                # half-chunks (4KB-contiguous descriptors, finer pipelining).
                # j1 rides the scalar queue right behind the t=0 x slices so
                # it lands before the PE finishes chewing j0; j2/j3 follow j0
                # on the sync queue; w_hh (not needed until t=1, ~50us in)
                # queues last on scalar.
                h2 = KI // 2
                for j in range(NJ):
                    wt = wp.tile([P, KI, 4 * P], io_dt, tag=f"wihf{j}")
                    if j == 0:
                        for k in range(KI):
                            nc.sync.dma_start(out=wt[:, k, :],
                                              in_=w_ih[:, 0, k, :])
                    else:
                        eng = nc.scalar if j in (1, 2) else nc.sync
                        eng.dma_start(out=wt[:, :h2, :],
                                      in_=w_ih[:, j, :h2, :])
                        eng.dma_start(out=wt[:, h2:, :],
                                      in_=w_ih[:, j, h2:, :])
                    w_ih_sb.append(wt)
                for j in range(NJ):
                    wt = wp.tile([P, KH, 4 * P], io_dt, tag=f"whh{j}")
                    nc.scalar.dma_start(out=wt, in_=w_hh[:, j, :, :])
                    w_hh_sb.append(wt)
            # prefetch t=1's x on the gpsimd engine's queue
            xt1 = None
            if T > 1:
                xt1 = xp.tile([P, KI, Bc], io_dt, tag="x")
                nc.gpsimd.dma_start(out=xt1, in_=xT[1])

            # h0 = c0 = 0, so step 0 skips the recurrent matmuls and the
            # f*c term entirely -- no initial state tiles needed (memset
            # can't produce float32r anyway).
            h_cur, c_cur = [], []

            GATE_FUNCS = (AF.Sigmoid, AF.Sigmoid, AF.Tanh, AF.Sigmoid)

            for t in range(T):
                if t == 0:
                    xt = xt0
                elif t == 1:
                    xt = xt1
                else:
                    xt = xp.tile([P, KI, Bc], io_dt, tag="x")
                    nc.sync.dma_start(out=xt, in_=xT[t])

                h_next, c_next = [], []
                for j in range(NJ):
                    acts = {}
                    # t=0: c0=0 makes the f-gate's contribution f*c0 vanish,
                    # so its whole matmul chain + sigmoid are skipped
                    for gi in ((0, 2, 3) if t == 0 else range(4)):
                        m = gi * NJ + j
                        ps = pp.tile([P, Bc], dt.float32, tag=f"ps{gi}")
                        for k in range(KI):
                            nc.tensor.matmul(
                                ps, lhsT=w_ih_sb[j][:, k, gi * P:(gi + 1) * P],
                                rhs=xt[:, k, :],
                                start=(k == 0),
                                stop=(t == 0 and k == KI - 1))
                        if t > 0:
                            for k in range(KH):
                                nc.tensor.matmul(
                                    ps, lhsT=w_hh_sb[j][:, k, gi * P:(gi + 1) * P],
                                    rhs=h_cur[k],
                                    start=False, stop=(k == KH - 1))
                        gt = wk.tile([P, Bc], dt.float32, tag=f"g{gi}",
                                     bufs=2)
                        nc.scalar.activation(gt, ps, GATE_FUNCS[gi],
                                             bias=bias_sb[:, m:m + 1])
                        acts[gi] = gt
                    i_t, g_t, o_t = acts[0], acts[2], acts[3]
                    cn = wk.tile([P, Bc], dt.float32, tag=f"c{j}")
                    if t == 0:
                        nc.vector.tensor_mul(cn, i_t, g_t)
                    else:
                        f_t = acts[1]
                        u = wk.tile([P, Bc], dt.float32, tag="u")
                        nc.vector.tensor_mul(u, i_t, g_t)
                        v = wk.tile([P, Bc], dt.float32, tag="v")
                        nc.vector.tensor_mul(v, f_t, c_cur[j])
                        nc.vector.tensor_add(cn, u, v)
                    th = wk.tile([P, Bc], dt.float32, tag="th")
                    nc.scalar.activation(th, cn, AF.Tanh)
                    hn = wk.tile([P, Bc], io_dt, tag=f"h{j}", bufs=2)
                    nc.vector.tensor_mul(hn, o_t, th)
                    if y_copy:
                        yh = wk.tile([P, Bc], dt.float16, tag="yh")
                        nc.vector.tensor_copy(yh, hn)
                        nc.sync.dma_start(out=yT[t, j * P:(j + 1) * P, :],
                                          in_=yh)
                    else:
                        nc.sync.dma_start(out=yT[t, j * P:(j + 1) * P, :],
                                          in_=hn)
                    h_next.append(hn)
                    c_next.append(cn)
                h_cur, c_cur = h_next, c_next

    nc.compile()
    return nc


def _get_nc(T=FRAME_LENGTH, Bc=BC, mode=MM_MODE, w16=W16, y16=Y16):
    key = (T, Bc, mode, w16, y16)
    if key not in _CACHE:
        _CACHE[key] = _build(T, Bc, mode, w16, y16)
    return _CACHE[key]


def _prep_inputs(embed_feats, w_ih_l, w_hh_l, b_ih_l, b_hh_l,
                 w_ih_r, w_hh_r, b_ih_r, b_hh_r, mode, w16):
    import ml_dtypes

    io_np = {"bf16": ml_dtypes.bfloat16,
             "fp16": np.float16}.get(mode, np.float32)
    w_np = np.float16 if (w16 and mode == "f32r") else io_np
    T = embed_feats.shape[1]

    w = {
        0: (np.asarray(w_ih_l), np.asarray(w_hh_l),
            np.asarray(b_ih_l) + np.asarray(b_hh_l)),
        1: (np.asarray(w_ih_r), np.asarray(w_hh_r),
            np.asarray(b_ih_r) + np.asarray(b_hh_r)),
    }
    x = np.asarray(embed_feats)

    # j-major column permutation of the 4H gate dim: block j holds the four
    # gates' columns for hidden chunk j, so each j-chunk loads contiguously
    j_idx, g_idx, c_idx = np.meshgrid(
        np.arange(NJ), np.arange(4), np.arange(P), indexing="ij")
    perm = (g_idx * (NJ * P) + j_idx * P + c_idx).reshape(-1)

    def dev_w(wmat, kt):
        # [IN, 4H] -> j-major column perm -> [P, NJ, kt, 4P] (the on-chip
        # tile layout, so each j-chunk DMA is fully contiguous per partition)
        wT = wmat.T[:, perm]                      # [IN, 4H] j-major cols
        wr = wT.reshape(kt, P, NJ, 4 * P)         # (k p) rows
        return np.ascontiguousarray(wr.transpose(1, 2, 0, 3)).astype(w_np)

    in_maps = []
    for c in range(NCORES):
        d, s = c // NSHARD, c % NSHARD
        xs = x[s * BC:(s + 1) * BC]
        if d == 1:
            xs = xs[:, ::-1]
        # [Bc, T, IN] -> [T, P, KI, Bc] (on-chip step-tile layout)
        xT = np.ascontiguousarray(
            xs.transpose(1, 2, 0).reshape(T, KI, P, BC)
            .transpose(0, 2, 1, 3)).astype(io_np)
        bias = np.ascontiguousarray(
            w[d][2].astype(np.float32).reshape(NM, P).T)
        in_maps.append({"xT": xT, "w_ih": dev_w(w[d][0], KI),
                        "w_hh": dev_w(w[d][1], KH), "bias": bias})
    return in_maps, T


def _run(inputs, mode=MM_MODE, trace=False, trace_kwargs=None):
    from concourse.bass_utils import run_bass_kernel_spmd

    in_maps, T = _prep_inputs(mode=mode, w16=W16, **inputs)
    nc = _get_nc(T=T, mode=mode)
    res = run_bass_kernel_spmd(nc, in_maps, list(range(NCORES)),
                               trace=trace, **(trace_kwargs or {}))

    out = np.empty((BATCH, T, 2 * HIDDEN), np.float32)
    for c in range(NCORES):
        d, s = c // NSHARD, c % NSHARD
        yt = np.asarray(res.results[c]["yT"], dtype=np.float32)  # [T, H, Bc]
        arr = yt.transpose(2, 0, 1)                              # [Bc, T, H]
        if d == 1:
            arr = arr[:, ::-1]
        out[s * BC:(s + 1) * BC, :, d * HIDDEN:(d + 1) * HIDDEN] = arr
    return out, res


def kernel(**inputs):
    out, _ = _run(inputs)
    return out
